# revision 36
# baseline (speedup 1.0000x reference)
"""Trainium2 Bass kernel for nn_MeanDegConv (gnn_message_passing) on 8 NeuronCores.

v3: merged stage-1 A/B sweep (xe finalizes per window), split AllGather so
stage-2 half-1 gathers overlap stage-1's tail, 4 SWDGE queues, bf16 gather
tables and matmuls, batched tensor_tensor one-hot builds (avoids the DVE
2-port perf-mode lockout of GpSimd SWDGE), ACT-only PSUM flushes.

Self-contained: imports the Bass/Tile stack from /opt/trn_rl_repo (part of the
container environment) and hardcodes all shapes/sharding for the problem.
"""
import sys
for _p in ('/opt/trn_rl_repo',):
    if _p not in sys.path:
        sys.path.insert(0, _p)

import bisect
import numpy as np
import ml_dtypes

import concourse.bass as bass
import concourse.mybir as mybir
import concourse.tile as tile
import concourse.bacc as bacc
from concourse.bass_utils import run_bass_kernel_spmd

N, E, NNZ, D = 50000, 10000, 1000000, 128
C = 8
EPC, VPC = E // C, N // C          # 1250 edges, 6250 vertices per core
NWIN_E = (EPC + 127) // 128        # 10
NWIN_V = (VPC + 127) // 128        # 49
EP = NWIN_E * 128                  # 1280 padded edge slots per core
HEP = EP // 2                      # 640: half the edge slots (windows 0-4)
VP = NWIN_V * 128                  # 6272 padded vertex slots per core
SPLIT = 32768                      # int16 index limit for the X table
NQ = 4                             # SWDGE queues
OH_BATCH = 8                       # one-hot tiles built per DVE instruction

F32 = mybir.dt.float32
BF16 = mybir.dt.bfloat16
I16 = mybir.dt.int16
BF = ml_dtypes.bfloat16


def _pack_idx16(idx32: np.ndarray) -> np.ndarray:
    """[L] int32 -> [128, L/16] int16 in the dma_gather wrap layout."""
    L = len(idx32)
    assert L % 16 == 0
    a = idx32.astype(np.int16).reshape(L // 16, 16).T  # [16, L/16]
    return np.ascontiguousarray(np.tile(a, (8, 1)))    # [128, L/16]


def _pad_to(arr, L, fill):
    out = np.full(L, fill, arr.dtype)
    out[:len(arr)] = arr
    return out


def _build_stream(per_win_idx, per_win_lidx, tiles_per_win):
    """Concatenate per-window (idx, lidx) entries, padding each window to
    tiles_per_win[w]*128 entries (idx pad 0, lidx pad -1)."""
    idx_parts, lidx_parts = [], []
    for w, T in enumerate(tiles_per_win):
        L = T * 128
        idx_parts.append(_pad_to(per_win_idx[w], L, 0))
        lidx_parts.append(_pad_to(per_win_lidx[w], L, -1.0))
    idx = np.concatenate(idx_parts) if idx_parts else np.zeros(0, np.int32)
    lidx = np.concatenate(lidx_parts) if lidx_parts else np.zeros(0, np.float32)
    assert len(idx) % 128 == 0
    return idx, lidx


def _chunk_sizes(ntiles, first_sizes, body, last_sizes=()):
    """Tile counts per chunk: first_sizes, then body-size chunks, with the
    final chunks tapered to last_sizes (so the drain tail is short)."""
    out = []
    left = ntiles
    for s in first_sizes:
        if left <= 0:
            break
        t = min(s, left)
        out.append(t)
        left -= t
    tail = []
    for s in last_sizes:
        if left - s <= 0:
            break
        tail.append(s)
        left -= s
    while left > 0:
        t = min(body, left)
        out.append(t)
        left -= t
    out.extend(reversed(tail))
    return out


def _plan(TA, TB, sizesA, sizesB, bodyA, bodyB, lastA=(), lastB=()):
    """Chunk plan for a two-stream merged pass.

    Returns (chunksA, chunksB, order) where chunksX = [(start_tile, ntiles)]
    and order = list of ('A'|'B', chunk_index) sorted by first consumption."""
    prefA = np.concatenate([[0], np.cumsum(TA)])
    prefB = np.concatenate([[0], np.cumsum(TB)])
    cA = _chunk_sizes(int(prefA[-1]), sizesA, bodyA, lastA)
    cB = _chunk_sizes(int(prefB[-1]), sizesB, bodyB, lastB)
    chunksA, s = [], 0
    for n in cA:
        chunksA.append((s, n)); s += n
    chunksB, s = [], 0
    for n in cB:
        chunksB.append((s, n)); s += n
    items = []
    for i, (st, n) in enumerate(chunksA):
        w = int(np.searchsorted(prefA, st, 'right')) - 1
        rank = st + int(prefB[min(w, len(TB))])
        items.append((rank, 0, 'A', i))
    for i, (st, n) in enumerate(chunksB):
        w = int(np.searchsorted(prefB, st, 'right')) - 1
        rank = int(prefA[min(w + 1, len(TA))]) + st
        items.append((rank, 1, 'B', i))
    items.sort()
    order = [(s, i) for _, _, s, i in items]
    return chunksA, chunksB, order


def prepare(inputs):
    """Host-side preprocessing: consts, per-core streams, schedule."""
    X = np.asarray(inputs["X"], np.float32)
    X0 = np.asarray(inputs["X0"], np.float32)
    v = np.asarray(inputs["vertex"]).astype(np.int64)
    e = np.asarray(inputs["edges"]).astype(np.int64)
    W1_w = np.asarray(inputs["W1_w"], np.float32); W1_b = np.asarray(inputs["W1_b"], np.float32)
    W2_w = np.asarray(inputs["W2_w"], np.float32); W2_b = np.asarray(inputs["W2_b"], np.float32)
    W3_w1 = np.asarray(inputs["W3_w1"], np.float32); W3_b1 = np.asarray(inputs["W3_b1"], np.float32)
    W3_w2 = np.asarray(inputs["W3_w2"], np.float32); W3_b2 = np.asarray(inputs["W3_b2"], np.float32)

    deg_e = np.bincount(e, minlength=E).astype(np.float32)
    deg_v = np.bincount(v, minlength=N).astype(np.float32)

    # ---- folded weight matrices (float64 for accuracy, cast at the end)
    W2a = W2_w[:D].astype(np.float64); W2b1 = W2_w[D:2*D].astype(np.float64)
    w2b_log = W2_w[2*D].astype(np.float64)
    R1 = W3_w1[:D].astype(np.float64); R2 = W3_w1[D:2*D].astype(np.float64)
    R3 = W3_w1[2*D:3*D].astype(np.float64); r4 = W3_w1[3*D].astype(np.float64)
    W2bR = W2b1 @ R1
    K1 = (W1_w.astype(np.float64) @ W2bR).astype(np.float32)
    k2 = (w2b_log @ R1).astype(np.float32)
    c1 = (W1_b.astype(np.float64) @ W2bR).astype(np.float32)
    MX = (W2a @ R1 + R2).astype(np.float32)
    MX0 = R3.astype(np.float32)
    c0 = (W2_b.astype(np.float64) @ R1 + W3_b1).astype(np.float32)

    consts = {
        "iota": np.ascontiguousarray(
            np.tile(np.arange(128, dtype=np.float32), (128, 1))).astype(BF),
        "K1": K1,
        "K2": np.ascontiguousarray(np.stack([k2, c1])),            # [2,128]
        "MX": MX.astype(BF), "MX0": MX0.astype(BF),
        "RC2": np.ascontiguousarray(np.stack([r4.astype(np.float32), c0])).astype(BF),
        "W3w2": W3_w2.astype(BF),
        "b2row": W3_b2.reshape(1, D).astype(BF),
        "ones1": np.ones((1, 128), BF),
        "Xtab": X.astype(BF),                                      # gather table bf16
    }

    # ---- stage-1: per (core, window, half) incidence lists
    core1 = (e // EPC).astype(np.int64)
    win1 = ((e % EPC) // 128).astype(np.int64)
    lidx1 = ((e % EPC) % 128).astype(np.float32)
    half1 = (v >= SPLIT).astype(np.int64)

    key1 = (core1 * NWIN_E + win1) * 2 + half1
    order1 = np.argsort(key1, kind="stable")
    ks = key1[order1]
    bounds1 = np.searchsorted(ks, np.arange(C * NWIN_E * 2 + 1))

    def seg1(c, w, h):
        b = (c * NWIN_E + w) * 2 + h
        return order1[bounds1[b]:bounds1[b + 1]]

    cnt1 = np.diff(bounds1).reshape(C, NWIN_E, 2)
    TA = [int(np.ceil(cnt1[:, w, 0].max() / 128)) for w in range(NWIN_E)]
    TB = [int(np.ceil(cnt1[:, w, 1].max() / 128)) for w in range(NWIN_E)]

    # ---- stage-2: per (core, window) lists, indices are padded xe row ids
    core2 = (v // VPC).astype(np.int64)
    win2 = ((v % VPC) // 128).astype(np.int64)
    lidx2 = ((v % VPC) % 128).astype(np.float32)
    rowid2 = (e // EPC) * EP + (e % EPC)         # padded row in xe_all

    key2 = core2 * NWIN_V + win2
    order2 = np.argsort(key2, kind="stable")
    ks2 = key2[order2]
    bounds2 = np.searchsorted(ks2, np.arange(C * NWIN_V + 1))

    def seg2(c, w):
        b = c * NWIN_V + w
        return order2[bounds2[b]:bounds2[b + 1]]

    cnt2 = np.diff(bounds2).reshape(C, NWIN_V)
    T2 = [int(np.ceil(cnt2[:, w].max() / 128)) for w in range(NWIN_V)]

    sched = {"TA": TA, "TB": TB, "T2": T2}

    # ---- per-core input maps
    in_maps = []
    log_deg_e = np.log(deg_e); log_deg_v = np.log(deg_v)
    for c in range(C):
        idxA = [v[seg1(c, w, 0)].astype(np.int32) for w in range(NWIN_E)]
        lidA = [lidx1[seg1(c, w, 0)] for w in range(NWIN_E)]
        idxB = [(v[seg1(c, w, 1)] - SPLIT).astype(np.int32) for w in range(NWIN_E)]
        lidB = [lidx1[seg1(c, w, 1)] for w in range(NWIN_E)]
        sA_idx, sA_lid = _build_stream(idxA, lidA, TA)
        sB_idx, sB_lid = _build_stream(idxB, lidB, TB)
        i2 = [rowid2[seg2(c, w)].astype(np.int32) for w in range(NWIN_V)]
        l2 = [lidx2[seg2(c, w)] for w in range(NWIN_V)]
        s2_idx, s2_lid = _build_stream(i2, l2, T2)

        de = np.ones(EP, np.float32); de[:EPC] = deg_e[c*EPC:(c+1)*EPC]
        le = np.zeros(EP, np.float32); le[:EPC] = log_deg_e[c*EPC:(c+1)*EPC]
        auxe = np.ascontiguousarray(np.stack([de * le, de]))        # [2, EP]
        invdeg_e_col = np.ascontiguousarray(
            (1.0 / de).reshape(NWIN_E, 128).T)                      # [128, NWIN_E]

        dv = np.ones(VP, np.float32); dv[:VPC] = deg_v[c*VPC:(c+1)*VPC]
        lv = np.zeros(VP, np.float32); lv[:VPC] = log_deg_v[c*VPC:(c+1)*VPC]
        auxv = np.ascontiguousarray(
            np.stack([lv, np.ones(VP, np.float32)])).astype(BF)     # [2, VP]
        invdeg_bc = np.ascontiguousarray(
            np.tile(1.0 / dv, (128, 1)))                            # [128, VP] f32

        Xp = np.zeros((VP, D), np.float32); Xp[:VPC] = X[c*VPC:(c+1)*VPC]
        X0p = np.zeros((VP, D), np.float32); X0p[:VPC] = X0[c*VPC:(c+1)*VPC]

        def packlid(lid):
            return np.ascontiguousarray(lid.reshape(-1, 128).T).astype(BF)

        m = dict(consts)
        m.update({
            "idxA": _pack_idx16(sA_idx), "lidA": packlid(sA_lid),
            "idxB": _pack_idx16(sB_idx), "lidB": packlid(sB_lid),
            "idx2": _pack_idx16(s2_idx), "lid2": packlid(s2_lid),
            "auxe": auxe, "invdeg_e_col": invdeg_e_col,
            "auxv": auxv, "invdeg_bc": invdeg_bc,
            "XT": np.ascontiguousarray(Xp.T).astype(BF),
            "X0T": np.ascontiguousarray(X0p.T).astype(BF),
        })
        in_maps.append(m)
    return in_maps, sched


def _bcast_cols(src_ap, ncols, width=128):
    """[128, ncols] AP -> [128, ncols, width] with each column broadcast."""
    return bass.AP(src_ap.tensor, src_ap.offset,
                   [src_ap.ap[0], src_ap.ap[1], [0, width]])


def _repeat_free(src_ap, reps):
    """[128, W] AP -> [128, reps, W] repeating the free dim."""
    return bass.AP(src_ap.tensor, src_ap.offset,
                   [src_ap.ap[0], [0, reps], src_ap.ap[1]])


class Stream:
    """Device-side view of one gather stream (idx/lid tiles + chunk table)."""

    def __init__(self, nc, name, idx_t, lid_t, table_ap, chunks, pool, tpc_max):
        self.nc, self.name = nc, name
        self.idx_t, self.lid_t, self.table_ap = idx_t, lid_t, table_ap
        self.chunks = chunks            # [(start_tile, ntiles)]
        self.starts = [s for s, _ in chunks]
        self.pool, self.tpc_max = pool, tpc_max
        self.g = [None] * len(chunks)
        self.p8 = {}

    def issue(self, ci, queue, iota_t, ohp):
        st, nt = self.chunks[ci]
        g = self.pool.tile([128, self.tpc_max, D], BF16, tag=self.pool.name + "_g")
        self.nc.gpsimd.dma_gather(
            out_ap=g[:, 0:nt, :],
            in_ap=self.table_ap,
            idxs_ap=self.idx_t[:, st * 8:(st + nt) * 8],
            num_idxs=nt * 128,
            num_idxs_reg=nt * 128,
            single_packet=False,
            elem_size=D,
            queue_num=queue,
        )
        self.g[ci] = g

    def tile_slices(self, ti, iota_t, ohp):
        """Return (g_slice, p_slice) for stream tile ti, lazily building the
        one-hot batch covering it."""
        ci = bisect.bisect_right(self.starts, ti) - 1
        st, nt = self.chunks[ci]
        g_sl = self.g[ci][:, ti - st, :]
        b0 = (ti // OH_BATCH) * OH_BATCH
        if b0 not in self.p8:
            nb = min(OH_BATCH, self.ntiles - b0)
            p8 = ohp.tile([128, OH_BATCH, 128], BF16, tag="p8")
            self.nc.vector.tensor_tensor(
                out=p8[:, 0:nb, :],
                in0=_repeat_free(iota_t[:], nb),
                in1=_bcast_cols(self.lid_t[:, b0:b0 + nb], nb),
                op=mybir.AluOpType.is_equal)
            self.p8[b0] = p8
        return g_sl, self.p8[b0][:, ti % OH_BATCH, :]


def build(in_map0, sched, mode="full"):
    """Build the SPMD Bass program. in_map0 supplies shapes."""
    TA, TB, T2 = sched["TA"], sched["TB"], sched["T2"]
    nc = bacc.Bacc(None, num_swdge_queues=NQ)

    def param(name, dt=F32):
        arr = in_map0[name]
        return nc.declare_dram_parameter(name, list(arr.shape), dt, isOutput=False)

    Xtab_d = param("Xtab", BF16)
    iota_d = param("iota", BF16); K1_d = param("K1"); K2_d = param("K2")
    MX_d = param("MX", BF16); MX0_d = param("MX0", BF16); RC2_d = param("RC2", BF16)
    W3w2_d = param("W3w2", BF16); b2row_d = param("b2row", BF16)
    ones1_d = param("ones1", BF16)
    idxA_d = param("idxA", I16); lidA_d = param("lidA", BF16)
    idxB_d = param("idxB", I16); lidB_d = param("lidB", BF16)
    idx2_d = param("idx2", I16); lid2_d = param("lid2", BF16)
    auxe_d = param("auxe"); invde_d = param("invdeg_e_col")
    auxv_d = param("auxv", BF16); invbc_d = param("invdeg_bc")
    XT_d = param("XT", BF16); X0T_d = param("X0T", BF16)
    out_d = nc.declare_dram_parameter("out", [VP, D], F32, isOutput=True)

    # chunk plans (host)
    c1A, c1B, order1 = _plan(TA, TB, [16, 32], [16], 48, 48,
                             lastA=(16, 32), lastB=(16,))
    nt2 = sum(T2)
    c2 = []
    s = 0
    for n in _chunk_sizes(nt2, [16, 32], 48, (16, 32)):
        c2.append((s, n)); s += n

    # queue 0 shares its Q7 pair with the engine's dispatch path: a q0 gather
    # holds the engine for its whole DGE, stalling dispatch to queues 1-3.
    # Route only small chunks there; big chunks round-robin on 1-3.
    qctr = [0]

    def next_q(ntiles=64):
        if ntiles <= 32:
            return 0
        q = 1 + qctr[0] % 3
        qctr[0] += 1
        return q

    with tile.TileContext(nc) as tc:
        with (
            tc.tile_pool(name="const", bufs=1) as cp,
            tc.tile_pool(name="stream", bufs=1) as sp,
            tc.tile_pool(name="gA", bufs=6) as gpA,
            tc.tile_pool(name="gB", bufs=2) as gpB,
            tc.tile_pool(name="oh", bufs=5) as ohp,
            tc.tile_pool(name="work", bufs=3) as wp,
            tc.tile_pool(name="psS", bufs=2, space="PSUM") as psS,
            tc.tile_pool(name="psXE", bufs=1, space="PSUM") as psXE,
            tc.tile_pool(name="psT", bufs=2, space="PSUM") as psT,
            tc.tile_pool(name="psR", bufs=2, space="PSUM") as psR,
            tc.tile_pool(name="psO", bufs=1, space="PSUM") as psO,
            tc.tile_pool(name="dram", bufs=1, space="DRAM") as dp,
        ):
            def load(pool, dram_ap, name, dt=F32):
                t = pool.tile(list(dram_ap.shape), dt, name=name, tag=name)
                nc.sync.dma_start(t[:], dram_ap[:])
                return t

            # streams first so gathers can start ASAP
            idxA_t = load(sp, idxA_d, "idxA", I16); lidA_t = load(sp, lidA_d, "lidA", BF16)
            idxB_t = load(sp, idxB_d, "idxB", I16); lidB_t = load(sp, lidB_d, "lidB", BF16)
            # warmup gather: absorbs the Q7 library-load / first-use sync
            gwarm = wp.tile([128, 1, D], BF16, tag="gwarm", name="gwarm")
            nc.gpsimd.dma_gather(
                out_ap=gwarm[:], in_ap=Xtab_d[0:SPLIT, :],
                idxs_ap=idxA_t[:, 0:8], num_idxs=128, num_idxs_reg=128,
                single_packet=False, elem_size=D, queue_num=1)
            idx2_t = load(sp, idx2_d, "idx2", I16); lid2_t = load(sp, lid2_d, "lid2", BF16)
            iota_t = load(cp, iota_d, "iota", BF16)
            K1_t = load(cp, K1_d, "K1"); K2_t = load(cp, K2_d, "K2")
            MX_t = load(cp, MX_d, "MX", BF16); MX0_t = load(cp, MX0_d, "MX0", BF16)
            RC2_t = load(cp, RC2_d, "RC2", BF16)
            W3w2_t = load(cp, W3w2_d, "W3w2", BF16)
            b2row_t = load(cp, b2row_d, "b2row", BF16)
            ones1_t = load(cp, ones1_d, "ones1", BF16)
            auxe_t = load(cp, auxe_d, "auxe"); invde_t = load(cp, invde_d, "invde")
            auxv_t = load(cp, auxv_d, "auxv", BF16)

            xe_local = dp.tile([EP, D], BF16)
            xe_all = dp.tile([C * EP, D], BF16, addr_space="Shared")

            # psr rows [d_out, vslot] for all vertex windows, prefetched in
            # stage-1 (independent of stage-2 data); bf16 is plenty here.
            psr_acc = cp.tile([128, VP], BF16, name="psr_acc")

            def psr_window(w):
                sl = slice(w * 128, (w + 1) * 128)
                xt = wp.tile([128, 128], BF16, tag="xt", name="xt")
                x0t = wp.tile([128, 128], BF16, tag="x0t", name="x0t")
                nc.sync.dma_start(xt[:], XT_d[:, sl])
                nc.sync.dma_start(x0t[:], X0T_d[:, sl])
                psr = psR.tile([128, 128], F32, tag="r", name="psr")
                nc.tensor.matmul(psr[:], MX_t[:], xt[:], start=True, stop=False)
                nc.tensor.matmul(psr[:], MX0_t[:], x0t[:], start=False, stop=False)
                nc.tensor.matmul(psr[:], RC2_t[:], auxv_t[:, sl], start=False, stop=True)
                nc.scalar.copy(psr_acc[:, sl], psr[:])

            sA = Stream(nc, "A", idxA_t, lidA_t, Xtab_d[0:SPLIT, :], c1A, gpA, 48)
            sB = Stream(nc, "B", idxB_t, lidB_t, Xtab_d[SPLIT:N, :], c1B, gpB, 48)
            sA.ntiles = sum(TA); sB.ntiles = sum(TB)
            s2 = Stream(nc, "2", idx2_t, lid2_t, xe_all[:], c2, gpA, 48)
            s2.ntiles = nt2

            # ---- stage-1 gather issue (need-ordered interleave of A/B)
            for skey, ci in order1:
                st_obj = sA if skey == 'A' else sB
                st_obj.issue(ci, next_q(st_obj.chunks[ci][1]), iota_t, ohp)

            # ---- stage-1 consume: per window, A tiles then B tiles, one
            # PSUM chain; xe computed at window close.
            prefA = np.concatenate([[0], np.cumsum(TA)]).astype(int)
            prefB = np.concatenate([[0], np.cumsum(TB)]).astype(int)
            pref2 = np.concatenate([[0], np.cumsum(T2)]).astype(int)

            def xe_window(w, ps_tile):
                ps = psXE.tile([128, 128], F32, tag="xe")
                nc.tensor.matmul(ps[:], ps_tile, K1_t[:], start=True, stop=False)
                nc.tensor.matmul(ps[:], auxe_t[:, w * 128:(w + 1) * 128], K2_t[:],
                                 start=False, stop=True)
                xe_sb = wp.tile([128, D], BF16, tag="xe_sb")
                nc.scalar.activation(
                    out=xe_sb[:], in_=ps[:],
                    func=mybir.ActivationFunctionType.Copy,
                    scale=invde_t[:, w:w + 1])
                nc.sync.dma_start(xe_local[w * 128:(w + 1) * 128, :], xe_sb[:])

            # spread the 49 psr-window computations across stage-1's windows
            psr_sched = {w: [] for w in range(NWIN_E)}
            if mode == "full":
                for i in range(NWIN_V):
                    psr_sched[min(i * NWIN_E // NWIN_V, NWIN_E - 1)].append(i)

            for w in range(NWIN_E):
                seq = [(sA, prefA[w] + t) for t in range(TA[w])] + \
                      [(sB, prefB[w] + t) for t in range(TB[w])]
                ps = psS.tile([128, 128], F32, tag="s1", name=f"ps1_{w}")
                for k, (st_obj, ti) in enumerate(seq):
                    g_sl, p_sl = st_obj.tile_slices(ti, iota_t, ohp)
                    nc.tensor.matmul(ps[:], g_sl, p_sl,
                                     start=(k == 0), stop=(k == len(seq) - 1))
                for i in psr_sched[w]:
                    psr_window(i)
                # S^T for this window is in ps ([d, e-slot]); flush to SBUF so
                # xe's matmul (which needs it as SBUF lhsT) can read it.
                sfl = wp.tile([128, 128], F32, tag="sfl", name=f"sfl_{w}")
                nc.scalar.copy(sfl[:], ps[:])
                xe_window(w, sfl[:])
                if mode != "s1" and w == NWIN_E - 1:
                    nc.gpsimd.collective_compute(
                        "AllGather", mybir.AluOpType.bypass,
                        replica_groups=[list(range(C))],
                        ins=[xe_local.opt()], outs=[xe_all.opt()])
                    for ci in range(len(c2)):
                        s2.issue(ci, next_q(s2.chunks[ci][1]), iota_t, ohp)

            if mode == "s1":
                for w in range(NWIN_E):
                    xe_rd = wp.tile([128, D], BF16, tag="xe_rd", name="xe_rd")
                    nc.sync.dma_start(xe_rd[:], xe_local[w * 128:(w + 1) * 128, :])
                    o32 = wp.tile([128, D], F32, tag="o32", name="o32")
                    nc.scalar.copy(o32[:], xe_rd[:])
                    nc.sync.dma_start(out_d[w * 128:(w + 1) * 128, :], o32[:])
            if mode == "full":
                # ================= stage 2 =================
                def finish_window(w, psT_tile):
                    sl = slice(w * 128, (w + 1) * 128)
                    invbc = wp.tile([128, 128], F32, tag="invbc", name="invbc")
                    nc.sync.dma_start(invbc[:], invbc_d[:, sl])
                    pre = wp.tile([128, 128], F32, tag="pre", name="pre")
                    nc.vector.tensor_tensor(out=pre[:], in0=psT_tile[:],
                                            in1=invbc[:], op=mybir.AluOpType.mult)
                    nc.vector.tensor_tensor(out=pre[:], in0=pre[:],
                                            in1=psr_acc[:, sl],
                                            op=mybir.AluOpType.add)
                    relu = wp.tile([128, 128], BF16, tag="relu", name="relu")
                    nc.scalar.activation(out=relu[:], in_=pre[:],
                                         func=mybir.ActivationFunctionType.Relu)
                    pso = psO.tile([128, 128], F32, tag="o", name="pso")
                    nc.tensor.matmul(pso[:], relu[:], W3w2_t[:], start=True, stop=False)
                    nc.tensor.matmul(pso[:], ones1_t[:], b2row_t[:], start=False, stop=True)
                    o_sb = wp.tile([128, D], F32, tag="o_sb", name="o_sb")
                    nc.scalar.copy(o_sb[:], pso[:])
                    nc.sync.dma_start(out_d[w * 128:(w + 1) * 128, :], o_sb[:])

                for w in range(NWIN_V):
                    seq = [(s2, pref2[w] + t) for t in range(T2[w])]
                    ps = psT.tile([128, 128], F32, tag="t3", name=f"ps2_{w}")
                    for k, (st_obj, ti) in enumerate(seq):
                        g_sl, p_sl = st_obj.tile_slices(ti, iota_t, ohp)
                        nc.tensor.matmul(ps[:], g_sl, p_sl,
                                         start=(k == 0), stop=(k == len(seq) - 1))
                    finish_window(w, ps)

    nc.finalize()
    return nc


def run(trace=False, mode="full", **inputs):
    in_maps, sched = prepare(inputs)
    nc = build(in_maps[0], sched, mode=mode)
    res = run_bass_kernel_spmd(nc, in_maps, list(range(C)), trace=trace)
    out = np.concatenate([res.results[c]["out"][:VPC] for c in range(C)], axis=0)
    return out, res


def kernel(**inputs):
    """Harness entry point: full inputs in, full [N, D] float32 output."""
    out, _res = run(trace=False, mode="full", **inputs)
    return out.astype(np.float32)


# revision 37
# speedup vs baseline: 1.1084x; 1.1084x over previous
"""Trainium2 Bass kernel for nn_MeanDegConv (gnn_message_passing) on 8 NeuronCores.

v3: merged stage-1 A/B sweep (xe finalizes per window), split AllGather so
stage-2 half-1 gathers overlap stage-1's tail, 4 SWDGE queues, bf16 gather
tables and matmuls, batched tensor_tensor one-hot builds (avoids the DVE
2-port perf-mode lockout of GpSimd SWDGE), ACT-only PSUM flushes.

Self-contained: imports the Bass/Tile stack from /opt/trn_rl_repo (part of the
container environment) and hardcodes all shapes/sharding for the problem.
"""
import sys
for _p in ('/opt/trn_rl_repo',):
    if _p not in sys.path:
        sys.path.insert(0, _p)

import bisect
import numpy as np
import ml_dtypes

import concourse.bass as bass
import concourse.mybir as mybir
import concourse.tile as tile
import concourse.bacc as bacc
from concourse.bass_utils import run_bass_kernel_spmd

N, E, NNZ, D = 50000, 10000, 1000000, 128
C = 8
EPC, VPC = E // C, N // C          # 1250 edges, 6250 vertices per core
NWIN_E = (EPC + 127) // 128        # 10
NWIN_V = (VPC + 127) // 128        # 49
EP = NWIN_E * 128                  # 1280 padded edge slots per core
HEP = EP // 2                      # 640: half the edge slots (windows 0-4)
VP = NWIN_V * 128                  # 6272 padded vertex slots per core
SPLIT = 32768                      # int16 index limit for the X table
NQ = 4                             # SWDGE queues
OH_BATCH = 8                       # one-hot tiles built per DVE instruction

F32 = mybir.dt.float32
BF16 = mybir.dt.bfloat16
I16 = mybir.dt.int16
BF = ml_dtypes.bfloat16


def _pack_idx16(idx32: np.ndarray) -> np.ndarray:
    """[L] int32 -> [128, L/16] int16 in the dma_gather wrap layout."""
    L = len(idx32)
    assert L % 16 == 0
    a = idx32.astype(np.int16).reshape(L // 16, 16).T  # [16, L/16]
    return np.ascontiguousarray(np.tile(a, (8, 1)))    # [128, L/16]


def _pad_to(arr, L, fill):
    out = np.full(L, fill, arr.dtype)
    out[:len(arr)] = arr
    return out


def _build_stream(per_win_idx, per_win_lidx, tiles_per_win):
    """Concatenate per-window (idx, lidx) entries, padding each window to
    tiles_per_win[w]*128 entries (idx pad 0, lidx pad -1)."""
    idx_parts, lidx_parts = [], []
    for w, T in enumerate(tiles_per_win):
        L = T * 128
        idx_parts.append(_pad_to(per_win_idx[w], L, 0))
        lidx_parts.append(_pad_to(per_win_lidx[w], L, -1.0))
    idx = np.concatenate(idx_parts) if idx_parts else np.zeros(0, np.int32)
    lidx = np.concatenate(lidx_parts) if lidx_parts else np.zeros(0, np.float32)
    assert len(idx) % 128 == 0
    return idx, lidx


def _chunk_sizes(ntiles, first_sizes, body, last_sizes=()):
    """Tile counts per chunk: first_sizes, then body-size chunks, with the
    final chunks tapered to last_sizes (so the drain tail is short)."""
    out = []
    left = ntiles
    for s in first_sizes:
        if left <= 0:
            break
        t = min(s, left)
        out.append(t)
        left -= t
    tail = []
    for s in last_sizes:
        if left - s <= 0:
            break
        tail.append(s)
        left -= s
    while left > 0:
        t = min(body, left)
        out.append(t)
        left -= t
    out.extend(reversed(tail))
    return out


def _plan(TA, TB, sizesA, sizesB, bodyA, bodyB, lastA=(), lastB=()):
    """Chunk plan for a two-stream merged pass.

    Returns (chunksA, chunksB, order) where chunksX = [(start_tile, ntiles)]
    and order = list of ('A'|'B', chunk_index) sorted by first consumption."""
    prefA = np.concatenate([[0], np.cumsum(TA)])
    prefB = np.concatenate([[0], np.cumsum(TB)])
    cA = _chunk_sizes(int(prefA[-1]), sizesA, bodyA, lastA)
    cB = _chunk_sizes(int(prefB[-1]), sizesB, bodyB, lastB)
    chunksA, s = [], 0
    for n in cA:
        chunksA.append((s, n)); s += n
    chunksB, s = [], 0
    for n in cB:
        chunksB.append((s, n)); s += n
    items = []
    for i, (st, n) in enumerate(chunksA):
        w = int(np.searchsorted(prefA, st, 'right')) - 1
        rank = st + int(prefB[min(w, len(TB))])
        items.append((rank, 0, 'A', i))
    for i, (st, n) in enumerate(chunksB):
        w = int(np.searchsorted(prefB, st, 'right')) - 1
        rank = int(prefA[min(w + 1, len(TA))]) + st
        items.append((rank, 1, 'B', i))
    items.sort()
    order = [(s, i) for _, _, s, i in items]
    return chunksA, chunksB, order


def prepare(inputs):
    """Host-side preprocessing: consts, per-core streams, schedule."""
    X = np.asarray(inputs["X"], np.float32)
    X0 = np.asarray(inputs["X0"], np.float32)
    v = np.asarray(inputs["vertex"]).astype(np.int64)
    e = np.asarray(inputs["edges"]).astype(np.int64)
    W1_w = np.asarray(inputs["W1_w"], np.float32); W1_b = np.asarray(inputs["W1_b"], np.float32)
    W2_w = np.asarray(inputs["W2_w"], np.float32); W2_b = np.asarray(inputs["W2_b"], np.float32)
    W3_w1 = np.asarray(inputs["W3_w1"], np.float32); W3_b1 = np.asarray(inputs["W3_b1"], np.float32)
    W3_w2 = np.asarray(inputs["W3_w2"], np.float32); W3_b2 = np.asarray(inputs["W3_b2"], np.float32)

    deg_e = np.bincount(e, minlength=E).astype(np.float32)
    deg_v = np.bincount(v, minlength=N).astype(np.float32)

    # ---- folded weight matrices (float64 for accuracy, cast at the end)
    W2a = W2_w[:D].astype(np.float64); W2b1 = W2_w[D:2*D].astype(np.float64)
    w2b_log = W2_w[2*D].astype(np.float64)
    R1 = W3_w1[:D].astype(np.float64); R2 = W3_w1[D:2*D].astype(np.float64)
    R3 = W3_w1[2*D:3*D].astype(np.float64); r4 = W3_w1[3*D].astype(np.float64)
    W2bR = W2b1 @ R1
    K1 = (W1_w.astype(np.float64) @ W2bR).astype(np.float32)
    k2 = (w2b_log @ R1).astype(np.float32)
    c1 = (W1_b.astype(np.float64) @ W2bR).astype(np.float32)
    MX = (W2a @ R1 + R2).astype(np.float32)
    MX0 = R3.astype(np.float32)
    c0 = (W2_b.astype(np.float64) @ R1 + W3_b1).astype(np.float32)

    consts = {
        "iota": np.ascontiguousarray(
            np.tile(np.arange(128, dtype=np.float32), (128, 1))).astype(BF),
        "K1": K1,
        "K2": np.ascontiguousarray(np.stack([k2, c1])),            # [2,128]
        "MX": MX.astype(BF), "MX0": MX0.astype(BF),
        "RC2": np.ascontiguousarray(np.stack([r4.astype(np.float32), c0])).astype(BF),
        "W3w2": W3_w2.astype(BF),
        "b2row": W3_b2.reshape(1, D).astype(BF),
        "ones1": np.ones((1, 128), BF),
        "Xtab": X.astype(BF),                                      # gather table bf16
    }

    # ---- stage-1: per (core, window, half) incidence lists
    core1 = (e // EPC).astype(np.int64)
    win1 = ((e % EPC) // 128).astype(np.int64)
    lidx1 = ((e % EPC) % 128).astype(np.float32)
    half1 = (v >= SPLIT).astype(np.int64)

    key1 = (core1 * NWIN_E + win1) * 2 + half1
    order1 = np.argsort(key1, kind="stable")
    ks = key1[order1]
    bounds1 = np.searchsorted(ks, np.arange(C * NWIN_E * 2 + 1))

    def seg1(c, w, h):
        b = (c * NWIN_E + w) * 2 + h
        return order1[bounds1[b]:bounds1[b + 1]]

    cnt1 = np.diff(bounds1).reshape(C, NWIN_E, 2)
    TA = [int(np.ceil(cnt1[:, w, 0].max() / 128)) for w in range(NWIN_E)]
    TB = [int(np.ceil(cnt1[:, w, 1].max() / 128)) for w in range(NWIN_E)]

    # ---- stage-2: per (core, window) lists, indices are padded xe row ids
    core2 = (v // VPC).astype(np.int64)
    win2 = ((v % VPC) // 128).astype(np.int64)
    lidx2 = ((v % VPC) % 128).astype(np.float32)
    rowid2 = (e // EPC) * EP + (e % EPC)         # padded row in xe_all

    key2 = core2 * NWIN_V + win2
    order2 = np.argsort(key2, kind="stable")
    ks2 = key2[order2]
    bounds2 = np.searchsorted(ks2, np.arange(C * NWIN_V + 1))

    def seg2(c, w):
        b = c * NWIN_V + w
        return order2[bounds2[b]:bounds2[b + 1]]

    cnt2 = np.diff(bounds2).reshape(C, NWIN_V)
    T2 = [int(np.ceil(cnt2[:, w].max() / 128)) for w in range(NWIN_V)]

    sched = {"TA": TA, "TB": TB, "T2": T2}

    # ---- per-core input maps
    in_maps = []
    log_deg_e = np.log(deg_e); log_deg_v = np.log(deg_v)
    for c in range(C):
        idxA = [v[seg1(c, w, 0)].astype(np.int32) for w in range(NWIN_E)]
        lidA = [lidx1[seg1(c, w, 0)] for w in range(NWIN_E)]
        idxB = [(v[seg1(c, w, 1)] - SPLIT).astype(np.int32) for w in range(NWIN_E)]
        lidB = [lidx1[seg1(c, w, 1)] for w in range(NWIN_E)]
        sA_idx, sA_lid = _build_stream(idxA, lidA, TA)
        sB_idx, sB_lid = _build_stream(idxB, lidB, TB)
        i2 = [rowid2[seg2(c, w)].astype(np.int32) for w in range(NWIN_V)]
        l2 = [lidx2[seg2(c, w)] for w in range(NWIN_V)]
        s2_idx, s2_lid = _build_stream(i2, l2, T2)

        de = np.ones(EP, np.float32); de[:EPC] = deg_e[c*EPC:(c+1)*EPC]
        le = np.zeros(EP, np.float32); le[:EPC] = log_deg_e[c*EPC:(c+1)*EPC]
        auxe = np.ascontiguousarray(np.stack([de * le, de]))        # [2, EP]
        invdeg_e_col = np.ascontiguousarray(
            (1.0 / de).reshape(NWIN_E, 128).T)                      # [128, NWIN_E]

        dv = np.ones(VP, np.float32); dv[:VPC] = deg_v[c*VPC:(c+1)*VPC]
        lv = np.zeros(VP, np.float32); lv[:VPC] = log_deg_v[c*VPC:(c+1)*VPC]
        auxv = np.ascontiguousarray(
            np.stack([lv, np.ones(VP, np.float32)])).astype(BF)     # [2, VP]
        invdeg_bc = np.ascontiguousarray(
            np.tile(1.0 / dv, (128, 1)))                            # [128, VP] f32

        Xp = np.zeros((VP, D), np.float32); Xp[:VPC] = X[c*VPC:(c+1)*VPC]
        X0p = np.zeros((VP, D), np.float32); X0p[:VPC] = X0[c*VPC:(c+1)*VPC]

        def packlid(lid):
            return np.ascontiguousarray(lid.reshape(-1, 128).T).astype(BF)

        m = dict(consts)
        m.update({
            "idxA": _pack_idx16(sA_idx), "lidA": packlid(sA_lid),
            "idxB": _pack_idx16(sB_idx), "lidB": packlid(sB_lid),
            "idx2": _pack_idx16(s2_idx), "lid2": packlid(s2_lid),
            "auxe": auxe, "invdeg_e_col": invdeg_e_col,
            "auxv": auxv, "invdeg_bc": invdeg_bc,
            "XT": np.ascontiguousarray(Xp.T).astype(BF),
            "X0T": np.ascontiguousarray(X0p.T).astype(BF),
        })
        in_maps.append(m)
    return in_maps, sched


def _bcast_cols(src_ap, ncols, width=128):
    """[128, ncols] AP -> [128, ncols, width] with each column broadcast."""
    return bass.AP(src_ap.tensor, src_ap.offset,
                   [src_ap.ap[0], src_ap.ap[1], [0, width]])


def _repeat_free(src_ap, reps):
    """[128, W] AP -> [128, reps, W] repeating the free dim."""
    return bass.AP(src_ap.tensor, src_ap.offset,
                   [src_ap.ap[0], [0, reps], src_ap.ap[1]])


class Stream:
    """Device-side view of one gather stream (idx/lid tiles + chunk table)."""

    def __init__(self, nc, name, idx_t, lid_t, table_ap, chunks, pool, tpc_max):
        self.nc, self.name = nc, name
        self.idx_t, self.lid_t, self.table_ap = idx_t, lid_t, table_ap
        self.chunks = chunks            # [(start_tile, ntiles)]
        self.starts = [s for s, _ in chunks]
        self.pool, self.tpc_max = pool, tpc_max
        self.g = [None] * len(chunks)
        self.p8 = {}

    def issue(self, ci, queue, iota_t, ohp):
        st, nt = self.chunks[ci]
        g = self.pool.tile([128, self.tpc_max, D], BF16, tag=self.pool.name + "_g")
        self.nc.gpsimd.dma_gather(
            out_ap=g[:, 0:nt, :],
            in_ap=self.table_ap,
            idxs_ap=self.idx_t[:, st * 8:(st + nt) * 8],
            num_idxs=nt * 128,
            num_idxs_reg=nt * 128,
            single_packet=False,
            elem_size=D,
            queue_num=queue,
        )
        self.g[ci] = g

    def tile_slices(self, ti, iota_t, ohp):
        """Return (g_slice, p_slice) for stream tile ti, lazily building the
        one-hot batch covering it."""
        ci = bisect.bisect_right(self.starts, ti) - 1
        st, nt = self.chunks[ci]
        g_sl = self.g[ci][:, ti - st, :]
        b0 = (ti // OH_BATCH) * OH_BATCH
        if b0 not in self.p8:
            nb = min(OH_BATCH, self.ntiles - b0)
            p8 = ohp.tile([128, OH_BATCH, 128], BF16, tag="p8")
            self.nc.vector.tensor_tensor(
                out=p8[:, 0:nb, :],
                in0=_repeat_free(iota_t[:], nb),
                in1=_bcast_cols(self.lid_t[:, b0:b0 + nb], nb),
                op=mybir.AluOpType.is_equal)
            self.p8[b0] = p8
        return g_sl, self.p8[b0][:, ti % OH_BATCH, :]


def build(in_map0, sched, mode="full"):
    """Build the SPMD Bass program. in_map0 supplies shapes."""
    TA, TB, T2 = sched["TA"], sched["TB"], sched["T2"]
    nc = bacc.Bacc(None, num_swdge_queues=NQ)

    def param(name, dt=F32):
        arr = in_map0[name]
        return nc.declare_dram_parameter(name, list(arr.shape), dt, isOutput=False)

    Xtab_d = param("Xtab", BF16)
    iota_d = param("iota", BF16); K1_d = param("K1"); K2_d = param("K2")
    MX_d = param("MX", BF16); MX0_d = param("MX0", BF16); RC2_d = param("RC2", BF16)
    W3w2_d = param("W3w2", BF16); b2row_d = param("b2row", BF16)
    ones1_d = param("ones1", BF16)
    idxA_d = param("idxA", I16); lidA_d = param("lidA", BF16)
    idxB_d = param("idxB", I16); lidB_d = param("lidB", BF16)
    idx2_d = param("idx2", I16); lid2_d = param("lid2", BF16)
    auxe_d = param("auxe"); invde_d = param("invdeg_e_col")
    auxv_d = param("auxv", BF16); invbc_d = param("invdeg_bc")
    XT_d = param("XT", BF16); X0T_d = param("X0T", BF16)
    out_d = nc.declare_dram_parameter("out", [VP, D], F32, isOutput=True)

    # chunk plans (host)
    c1A, c1B, order1 = _plan(TA, TB, [16, 32], [16], 48, 48,
                             lastA=(16, 32), lastB=(16,))
    nt2 = sum(T2)
    c2 = []
    s = 0
    for n in _chunk_sizes(nt2, [16, 32], 48, (16, 32)):
        c2.append((s, n)); s += n

    qctr = [0]
    QORDER = (1, 2, 3, 0)

    def next_q(ntiles=64):
        q = QORDER[qctr[0] % NQ]
        qctr[0] += 1
        return q

    with tile.TileContext(nc) as tc:
        with (
            tc.tile_pool(name="const", bufs=1) as cp,
            tc.tile_pool(name="stream", bufs=1) as sp,
            tc.tile_pool(name="gA", bufs=6) as gpA,
            tc.tile_pool(name="gB", bufs=2) as gpB,
            tc.tile_pool(name="oh", bufs=5) as ohp,
            tc.tile_pool(name="work", bufs=3) as wp,
            tc.tile_pool(name="psS", bufs=2, space="PSUM") as psS,
            tc.tile_pool(name="psXE", bufs=1, space="PSUM") as psXE,
            tc.tile_pool(name="psT", bufs=2, space="PSUM") as psT,
            tc.tile_pool(name="psR", bufs=2, space="PSUM") as psR,
            tc.tile_pool(name="psO", bufs=1, space="PSUM") as psO,
            tc.tile_pool(name="dram", bufs=1, space="DRAM") as dp,
        ):
            def load(pool, dram_ap, name, dt=F32):
                t = pool.tile(list(dram_ap.shape), dt, name=name, tag=name)
                nc.sync.dma_start(t[:], dram_ap[:])
                return t

            # streams first so gathers can start ASAP
            idxA_t = load(sp, idxA_d, "idxA", I16); lidA_t = load(sp, lidA_d, "lidA", BF16)
            idxB_t = load(sp, idxB_d, "idxB", I16); lidB_t = load(sp, lidB_d, "lidB", BF16)
            # warmup gather: absorbs the Q7 library-load / first-use sync
            gwarm = wp.tile([128, 1, D], BF16, tag="gwarm", name="gwarm")
            nc.gpsimd.dma_gather(
                out_ap=gwarm[:], in_ap=Xtab_d[0:SPLIT, :],
                idxs_ap=idxA_t[:, 0:8], num_idxs=128, num_idxs_reg=128,
                single_packet=False, elem_size=D, queue_num=1)
            idx2_t = load(sp, idx2_d, "idx2", I16); lid2_t = load(sp, lid2_d, "lid2", BF16)
            iota_t = load(cp, iota_d, "iota", BF16)
            K1_t = load(cp, K1_d, "K1"); K2_t = load(cp, K2_d, "K2")
            MX_t = load(cp, MX_d, "MX", BF16); MX0_t = load(cp, MX0_d, "MX0", BF16)
            RC2_t = load(cp, RC2_d, "RC2", BF16)
            W3w2_t = load(cp, W3w2_d, "W3w2", BF16)
            b2row_t = load(cp, b2row_d, "b2row", BF16)
            ones1_t = load(cp, ones1_d, "ones1", BF16)
            auxe_t = load(cp, auxe_d, "auxe"); invde_t = load(cp, invde_d, "invde")
            auxv_t = load(cp, auxv_d, "auxv", BF16)

            xe_local = dp.tile([EP, D], BF16)
            xe_all = dp.tile([C * EP, D], BF16, addr_space="Shared")

            # psr rows [d_out, vslot] for all vertex windows, prefetched in
            # stage-1 (independent of stage-2 data); bf16 is plenty here.
            psr_acc = cp.tile([128, VP], BF16, name="psr_acc")

            def psr_window(w):
                sl = slice(w * 128, (w + 1) * 128)
                xt = wp.tile([128, 128], BF16, tag="xt", name="xt")
                x0t = wp.tile([128, 128], BF16, tag="x0t", name="x0t")
                nc.sync.dma_start(xt[:], XT_d[:, sl])
                nc.sync.dma_start(x0t[:], X0T_d[:, sl])
                psr = psR.tile([128, 128], F32, tag="r", name="psr")
                nc.tensor.matmul(psr[:], MX_t[:], xt[:], start=True, stop=False)
                nc.tensor.matmul(psr[:], MX0_t[:], x0t[:], start=False, stop=False)
                nc.tensor.matmul(psr[:], RC2_t[:], auxv_t[:, sl], start=False, stop=True)
                nc.scalar.copy(psr_acc[:, sl], psr[:])

            sA = Stream(nc, "A", idxA_t, lidA_t, Xtab_d[0:SPLIT, :], c1A, gpA, 48)
            sB = Stream(nc, "B", idxB_t, lidB_t, Xtab_d[SPLIT:N, :], c1B, gpB, 48)
            sA.ntiles = sum(TA); sB.ntiles = sum(TB)
            s2 = Stream(nc, "2", idx2_t, lid2_t, xe_all[:], c2, gpA, 48)
            s2.ntiles = nt2

            # ---- stage-1 gather issue (need-ordered interleave of A/B)
            for skey, ci in order1:
                st_obj = sA if skey == 'A' else sB
                st_obj.issue(ci, next_q(st_obj.chunks[ci][1]), iota_t, ohp)

            # ---- stage-1 consume: per window, A tiles then B tiles, one
            # PSUM chain; xe computed at window close.
            prefA = np.concatenate([[0], np.cumsum(TA)]).astype(int)
            prefB = np.concatenate([[0], np.cumsum(TB)]).astype(int)
            pref2 = np.concatenate([[0], np.cumsum(T2)]).astype(int)

            def xe_window(w, ps_tile):
                ps = psXE.tile([128, 128], F32, tag="xe")
                nc.tensor.matmul(ps[:], ps_tile, K1_t[:], start=True, stop=False)
                nc.tensor.matmul(ps[:], auxe_t[:, w * 128:(w + 1) * 128], K2_t[:],
                                 start=False, stop=True)
                xe_sb = wp.tile([128, D], BF16, tag="xe_sb")
                nc.scalar.activation(
                    out=xe_sb[:], in_=ps[:],
                    func=mybir.ActivationFunctionType.Copy,
                    scale=invde_t[:, w:w + 1])
                nc.sync.dma_start(xe_local[w * 128:(w + 1) * 128, :], xe_sb[:])

            # spread the 49 psr-window computations across stage-1's windows
            psr_sched = {w: [] for w in range(NWIN_E)}
            if mode == "full":
                for i in range(NWIN_V):
                    psr_sched[min(i * NWIN_E // NWIN_V, NWIN_E - 1)].append(i)

            for w in range(NWIN_E):
                seq = [(sA, prefA[w] + t) for t in range(TA[w])] + \
                      [(sB, prefB[w] + t) for t in range(TB[w])]
                ps = psS.tile([128, 128], F32, tag="s1", name=f"ps1_{w}")
                for k, (st_obj, ti) in enumerate(seq):
                    g_sl, p_sl = st_obj.tile_slices(ti, iota_t, ohp)
                    nc.tensor.matmul(ps[:], g_sl, p_sl,
                                     start=(k == 0), stop=(k == len(seq) - 1))
                for i in psr_sched[w]:
                    psr_window(i)
                # S^T for this window is in ps ([d, e-slot]); flush to SBUF so
                # xe's matmul (which needs it as SBUF lhsT) can read it.
                sfl = wp.tile([128, 128], F32, tag="sfl", name=f"sfl_{w}")
                nc.scalar.copy(sfl[:], ps[:])
                xe_window(w, sfl[:])
                if mode != "s1" and w == NWIN_E - 1:
                    nc.gpsimd.collective_compute(
                        "AllGather", mybir.AluOpType.bypass,
                        replica_groups=[list(range(C))],
                        ins=[xe_local.opt()], outs=[xe_all.opt()])
                    for ci in range(len(c2)):
                        s2.issue(ci, next_q(s2.chunks[ci][1]), iota_t, ohp)

            if mode == "s1":
                for w in range(NWIN_E):
                    xe_rd = wp.tile([128, D], BF16, tag="xe_rd", name="xe_rd")
                    nc.sync.dma_start(xe_rd[:], xe_local[w * 128:(w + 1) * 128, :])
                    o32 = wp.tile([128, D], F32, tag="o32", name="o32")
                    nc.scalar.copy(o32[:], xe_rd[:])
                    nc.sync.dma_start(out_d[w * 128:(w + 1) * 128, :], o32[:])
            if mode == "full":
                # ================= stage 2 =================
                def finish_window(w, psT_tile):
                    sl = slice(w * 128, (w + 1) * 128)
                    invbc = wp.tile([128, 128], F32, tag="invbc", name="invbc")
                    nc.sync.dma_start(invbc[:], invbc_d[:, sl])
                    pre = wp.tile([128, 128], F32, tag="pre", name="pre")
                    nc.vector.tensor_tensor(out=pre[:], in0=psT_tile[:],
                                            in1=invbc[:], op=mybir.AluOpType.mult)
                    nc.vector.tensor_tensor(out=pre[:], in0=pre[:],
                                            in1=psr_acc[:, sl],
                                            op=mybir.AluOpType.add)
                    relu = wp.tile([128, 128], BF16, tag="relu", name="relu")
                    nc.scalar.activation(out=relu[:], in_=pre[:],
                                         func=mybir.ActivationFunctionType.Relu)
                    pso = psO.tile([128, 128], F32, tag="o", name="pso")
                    nc.tensor.matmul(pso[:], relu[:], W3w2_t[:], start=True, stop=False)
                    nc.tensor.matmul(pso[:], ones1_t[:], b2row_t[:], start=False, stop=True)
                    o_sb = wp.tile([128, D], F32, tag="o_sb", name="o_sb")
                    nc.scalar.copy(o_sb[:], pso[:])
                    nc.sync.dma_start(out_d[w * 128:(w + 1) * 128, :], o_sb[:])

                for w in range(NWIN_V):
                    seq = [(s2, pref2[w] + t) for t in range(T2[w])]
                    ps = psT.tile([128, 128], F32, tag="t3", name=f"ps2_{w}")
                    for k, (st_obj, ti) in enumerate(seq):
                        g_sl, p_sl = st_obj.tile_slices(ti, iota_t, ohp)
                        nc.tensor.matmul(ps[:], g_sl, p_sl,
                                         start=(k == 0), stop=(k == len(seq) - 1))
                    finish_window(w, ps)

    nc.finalize()
    return nc


def run(trace=False, mode="full", **inputs):
    in_maps, sched = prepare(inputs)
    nc = build(in_maps[0], sched, mode=mode)
    res = run_bass_kernel_spmd(nc, in_maps, list(range(C)), trace=trace)
    out = np.concatenate([res.results[c]["out"][:VPC] for c in range(C)], axis=0)
    return out, res


def kernel(**inputs):
    """Harness entry point: full inputs in, full [N, D] float32 output."""
    out, _res = run(trace=False, mode="full", **inputs)
    return out.astype(np.float32)
